# revision 1
# baseline (speedup 1.0000x reference)
"""DMPNN layer kernel for Trainium2, data-parallel over batch on 8 NeuronCores.

Math (reference):
    gate[i,j]  = (sum_b adj[b,i,j]) > 0                      [N,N], shared across batch
    hW[b,i,o]  = sum_c h[b,i,c] * Wh[o,c]                    Wh = W_w[:, :H]
    term_h     = sum_i gate[i,j] * hW[b,i,o]
    e_sum      = sum_i gate[i,j] * edge_attr[b,i,j,e]
    term_e     = sum_e e_sum[b,j,e] * We[o,e]                We = W_w[:, H:]
    count[j]   = sum_i gate[i,j]
    msg        = term_h + term_e + count[j]*W_b[o]
    msg       *= (j < num_nodes[b])
    h_new      = (h + msg) @ U_w.T + U_b

Per-core layout (feature-major "T" = [hidden_on_partitions, nodes_on_free]):
  - edge_attr streamed as [i_chunk=128, (j,e)=4096] tiles (contiguous rows),
    gated by a precomputed gate_bcast [i, j*16+e] mask (DVE), reduced over i
    by ones-vector matmuls into PSUM [8,512] -> flattened to e_sum [1,4096].
  - msgT [o=128, j=256] accumulated in one PSUM bank: 2 matmuls (term_h)
    + 1 outer product (bias) + 16 outer products (term_e, rank-1 per e).
  - xT = msgT*mask + hT; h_new chunks = xT_chunk.T @ U_wT + U_b.
  - gate computed on-device from the full adj (int8, all 32 batches on every
    core) by tree-reduction over b; no cross-core collective needed.
"""

import os
import sys

for _p in ("/opt/trn_rl_repo", "/root/.axon_site/_ro/trn_rl_repo"):
    if _p not in sys.path:
        sys.path.insert(0, _p)

import numpy as np

import concourse.bass as bass
import concourse.tile as tile
from concourse import bacc, mybir
from concourse.bass_utils import run_bass_kernel_spmd

B, N, H, E = 32, 256, 128, 16
N_CORES = 8
BL = B // N_CORES          # batches per core
NJE = N * E                # 4096
F32 = mybir.dt.float32
I8 = mybir.dt.int8


def build_nc(reps: int = 1, variant: str = "flat"):
    """variant: "flat"  - e_sum flattened to [1,4096], 16 rank-1 term_e mms
                "est"   - e_sum direct to [16,256] via strided-rhs reduce mms,
                          single k=16 term_e matmul
                "fast"  - est structure + float32r matmuls (tf32-like, 4x PE
                          rate for fp32 data) + Hadamard split DVE/GpSimd"""
    est_like = variant in ("est", "fast")
    fast = variant == "fast"
    F32R = mybir.dt.float32r
    # dtype for tensors that feed fp32r matmuls: their PRODUCER instruction
    # must write float32r (walrus verifier requires rounded inputs)
    CR = F32R if fast else F32

    def rcast(ap):
        return ap.bitcast(F32R) if fast else ap

    nc = bacc.Bacc("TRN2", target_bir_lowering=False, debug=False,
                   num_devices=N_CORES)

    d_h = nc.dram_tensor("h", [BL, N, H], F32, kind="ExternalInput")
    d_ea = nc.dram_tensor("ea", [BL, N, N, E], F32, kind="ExternalInput")
    # adj bit-packed host-side (lossless encoding): bit b of word [i, j] is
    # adj[b, i, j] != 0. The any-over-batch reduction happens on device as
    # a single word != 0 compare per element.
    d_adj = nc.dram_tensor("adjb", [N, N], mybir.dt.int32,
                           kind="ExternalInput")
    d_mask = nc.dram_tensor("mask", [BL, N], F32, kind="ExternalInput")
    d_ww = nc.dram_tensor("ww", [H, H + E], F32, kind="ExternalInput")
    d_wb = nc.dram_tensor("wb", [1, H], CR, kind="ExternalInput")
    d_uw = nc.dram_tensor("uw", [H, H], F32, kind="ExternalInput")
    d_ub = nc.dram_tensor("ub", [1, H], F32, kind="ExternalInput")
    d_ident = nc.dram_tensor("ident", [128, 128], F32, kind="ExternalInput")
    d_ones = nc.dram_tensor("ones", [128, 1], CR, kind="ExternalInput")
    d_sel8 = nc.dram_tensor("sel8", [128, 64], F32, kind="ExternalInput")
    d_sel16 = nc.dram_tensor("sel16", [128, 256], CR, kind="ExternalInput")
    d_y = nc.dram_tensor("y", [BL, N, H], F32, kind="ExternalOutput")

    with tile.TileContext(nc) as tc:
        with (
            tc.tile_pool(name="const", bufs=1) as cpool,
            tc.tile_pool(name="gatep", bufs=1) as gpool,
            tc.tile_pool(name="ea", bufs=4) as eapool,
            tc.tile_pool(name="work", bufs=2) as wpool,
            tc.tile_pool(name="ps_tr", bufs=1, space="PSUM") as ps_tr,
            tc.tile_pool(name="ps_es", bufs=2, space="PSUM") as ps_es,
            tc.tile_pool(name="ps_hw", bufs=1, space="PSUM") as ps_hw,
            tc.tile_pool(name="ps_msg", bufs=2, space="PSUM") as ps_msg,
            tc.tile_pool(name="ps_up", bufs=1, space="PSUM") as ps_up,
        ):
            # ---- constants -------------------------------------------------
            ident = cpool.tile([128, 128], F32)
            nc.sync.dma_start(ident[:], d_ident[:])
            ones = cpool.tile([128, 1], CR)
            nc.sync.dma_start(ones[:], d_ones[:])
            sel8 = cpool.tile([128, 64], F32)
            nc.sync.dma_start(sel8[:], d_sel8[:])
            sel16 = cpool.tile([128, 256], CR)
            nc.sync.dma_start(sel16[:], d_sel16[:])
            ww = cpool.tile([H, H + E], F32)
            nc.sync.dma_start(ww[:], d_ww[:])
            uw = cpool.tile([H, H], F32)
            nc.sync.dma_start(uw[:], d_uw[:])
            wb = cpool.tile([1, H], CR)
            nc.sync.dma_start(wb[:], d_wb[:])
            ub_row = cpool.tile([1, H], F32)
            nc.sync.dma_start(ub_row[:], d_ub[:])

            # transposes of the weight blocks (once)
            whT = cpool.tile([H, H], CR)       # [c, o] = Wh[o, c]
            weT = cpool.tile([E, H], CR)       # [e, o] = We[o, e]
            uwT = cpool.tile([H, H], CR)       # [c, o] = U_w[o, c]
            tr_ps = ps_tr.tile([128, 128], F32, name="tr")
            nc.tensor.transpose(tr_ps[:], ww[:, 0:H], ident[:])
            nc.scalar.copy(whT[:], tr_ps[:])
            tr_ps2 = ps_tr.tile([128, 128], F32, name="tr")
            nc.tensor.transpose(tr_ps2[:E, :], ww[:, H:H + E], ident[:])
            nc.scalar.copy(weT[:], tr_ps2[:E, :])
            if not est_like:
                # flatten weT rows to partition 0 so outer-product lhsT APs
                # have base partition 0 (PE requires base in {0, 32, 64})
                weT_f = cpool.tile([1, E * H], F32)
                for e in range(E):
                    nc.sync.dma_start(weT_f[0:1, bass.ts(e, H)],
                                      weT[e:e + 1, :])
            tr_ps3 = ps_tr.tile([128, 128], F32, name="tr")
            nc.tensor.transpose(tr_ps3[:], uw[:], ident[:])
            nc.scalar.copy(uwT[:], tr_ps3[:])

            ub_b = cpool.tile([128, H], F32)    # U_b broadcast over partitions
            nc.gpsimd.partition_broadcast(ub_b[:], ub_row[0:1, :])

            for rep in range(reps):
                # ---- gate from adj (all 32 batches, tree-reduce over b) ----
                gate = []      # per i-chunk: [128, N] f32 0/1
                gate_bc = []   # per i-chunk: [128, N*E] f32, gate[i,j] at j*16+e
                for c in range(2):
                    at = gpool.tile([128, N], mybir.dt.int32,
                                    name=f"adj_t{c}")
                    nc.sync.dma_start(at[:], d_adj[bass.ts(c, 128), :])
                    g = gpool.tile([128, N], CR, name=f"gate{c}")
                    nc.vector.tensor_scalar(g[:], at[:], 0, None,
                                            mybir.AluOpType.not_equal)
                    gb = gpool.tile([128, NJE], F32, name=f"gateb{c}")
                    gb_v = gb[:].rearrange("p (j e) -> p j e", e=E)
                    for e in range(E):
                        if fast:
                            nc.scalar.copy(gb_v[:, :, e], g[:])
                        else:
                            nc.gpsimd.tensor_copy(gb_v[:, :, e], g[:])
                    gate.append(g)
                    gate_bc.append(gb)

                # count[j] = sum_i gate[i, j]
                cnt_ps = ps_tr.tile([1, N], F32, name="tr")
                for c in range(2):
                    nc.tensor.matmul(cnt_ps[:], rcast(ones[:]),
                                     rcast(gate[c][:]),
                                     start=(c == 0), stop=(c == 1))
                cnt = cpool.tile([1, N], CR, name="cnt_sb")
                nc.scalar.copy(cnt[:], cnt_ps[:])

                for b in range(BL):
                    # ---- hT [c, i] -----------------------------------------
                    hT = wpool.tile([H, N], CR, name="hT")
                    for c in range(2):
                        hn = wpool.tile([128, H], F32, name="h_nat")
                        nc.sync.dma_start(hn[:], d_h[b, bass.ts(c, 128), :])
                        htp = ps_tr.tile([128, 128], F32, name="htp")
                        nc.tensor.transpose(htp[:], hn[:], ident[:])
                        nc.scalar.copy(hT[:, bass.ts(c, 128)], htp[:])

                    # ---- hW natural [i, o], both chunks in one psum bank ---
                    hw_ps = ps_hw.tile([128, 2 * H], F32, name="hw_ps")
                    for c in range(2):
                        nc.tensor.matmul(hw_ps[:, bass.ts(c, H)],
                                         rcast(hT[:, bass.ts(c, 128)]),
                                         rcast(whT[:]),
                                         start=True, stop=True)
                    hw = wpool.tile([128, 2 * H], CR, name="hw")
                    nc.scalar.copy(hw[:], hw_ps[:])

                    # ---- gated edge stream + i-reduction -------------------
                    if not est_like:
                        es_ps = ps_es.tile([8, 512], F32, name="es_ps")
                    else:
                        es_ps = ps_es.tile([E, N], F32, name="es_ps")
                    for c in range(2):
                        ea_t = eapool.tile([128, NJE], F32, name="ea_t")
                        nc.sync.dma_start(
                            ea_t[:],
                            d_ea[b, bass.ts(c, 128), :, :].rearrange(
                                "p j e -> p (j e)"))
                        had_eng = nc.gpsimd if (fast and c == 1) else nc.vector
                        if fast:
                            # separate f32r output tile: the verifier requires
                            # every writer of an fp32r-matmul operand to round
                            # to f32r (an in-place gating would leave the DMA
                            # as an unrounded writer of the same location)
                            gea = eapool.tile([128, NJE], F32R, name="gea")
                            had_eng.tensor_tensor(gea[:], ea_t[:],
                                                  gate_bc[c][:],
                                                  mybir.AluOpType.mult)
                        else:
                            gea = ea_t
                            had_eng.tensor_tensor(ea_t[:], ea_t[:],
                                                  gate_bc[c][:],
                                                  mybir.AluOpType.mult)
                        if not est_like:
                            for t in range(8):
                                # lhsT = sel8[:, t*8:(t+1)*8]: all-ones in
                                # column t -> row t of es_ps accumulates the
                                # i-partition sum of this 512-wide slice.
                                nc.tensor.matmul(es_ps[:, :],
                                                 sel8[:, bass.ts(t, 8)],
                                                 gea[:, bass.ts(t, 512)],
                                                 start=(c == 0 and t == 0),
                                                 stop=(c == 1 and t == 7))
                        else:
                            ea_v = gea[:].rearrange("p (j e) -> p j e", e=E)
                            for e in range(E):
                                # row e of es_ps accumulates sum_i of the
                                # stride-16 j-slice for attribute e
                                nc.tensor.matmul(es_ps[:, :],
                                                 rcast(sel16[:, bass.ts(e, E)]),
                                                 rcast(ea_v[:, :, e]),
                                                 start=(c == 0 and e == 0),
                                                 stop=(c == 1 and e == E - 1))
                    if not est_like:
                        es_sb = wpool.tile([8, 512], F32, name="es_sb")
                        nc.scalar.copy(es_sb[:], es_ps[:])
                        esf = wpool.tile([1, NJE], F32, name="esf")
                        for t in range(8):
                            nc.sync.dma_start(esf[:, bass.ts(t, 512)],
                                              es_sb[t:t + 1, :])
                        esf_v = esf[:].rearrange("p (j e) -> p j e", e=E)
                    else:
                        esT_sb = wpool.tile([E, N], CR, name="es_sb")
                        nc.scalar.copy(esT_sb[:], es_ps[:])

                    # ---- msgT [o, j] accumulation --------------------------
                    msg_ps = ps_msg.tile([H, N], F32, name="msg_ps")
                    for c in range(2):
                        nc.tensor.matmul(msg_ps[:], rcast(hw[:, bass.ts(c, H)]),
                                         rcast(gate[c][:]), start=(c == 0),
                                         stop=False)
                    nc.tensor.matmul(msg_ps[:], rcast(wb[:]), rcast(cnt[:]),
                                     start=False, stop=False)
                    if not est_like:
                        for e in range(E):
                            nc.tensor.matmul(msg_ps[:],
                                             weT_f[0:1, bass.ts(e, H)],
                                             esf_v[:, :, e], start=False,
                                             stop=(e == E - 1))
                    else:
                        nc.tensor.matmul(msg_ps[:], rcast(weT[:]),
                                         rcast(esT_sb[:]),
                                         start=False, stop=True)

                    # ---- mask + add h --------------------------------------
                    mrow = wpool.tile([1, N], F32, name="mrow")
                    nc.sync.dma_start(mrow[:], d_mask[b:b + 1, :])
                    maskb = wpool.tile([128, N], F32, name="maskb")
                    nc.gpsimd.partition_broadcast(maskb[:], mrow[0:1, :])
                    xT = wpool.tile([H, N], CR, name="xT")
                    nc.vector.tensor_tensor(xT[:], msg_ps[:], maskb[:],
                                            mybir.AluOpType.mult)
                    nc.vector.tensor_tensor(xT[:], xT[:], hT[:],
                                            mybir.AluOpType.add)

                    # ---- h_new = xT.T @ uwT + ub ---------------------------
                    up_ps = ps_up.tile([128, 2 * H], F32, name="up_ps")
                    for c in range(2):
                        nc.tensor.matmul(up_ps[:, bass.ts(c, H)],
                                         rcast(xT[:, bass.ts(c, 128)]),
                                         rcast(uwT[:]),
                                         start=True, stop=True)
                    yt = wpool.tile([128, 2 * H], F32, name="yt")
                    for c in range(2):
                        nc.vector.tensor_tensor(yt[:, bass.ts(c, H)],
                                                up_ps[:, bass.ts(c, H)],
                                                ub_b[:],
                                                mybir.AluOpType.add)
                    for c in range(2):
                        nc.sync.dma_start(d_y[b, bass.ts(c, 128), :],
                                          yt[:, bass.ts(c, H)])

    nc.compile()
    return nc


def _host_prep(h, edge_attr, adj, num_nodes):
    h = np.ascontiguousarray(np.asarray(h, dtype=np.float32))
    edge_attr = np.ascontiguousarray(np.asarray(edge_attr, dtype=np.float32))
    # bit-pack adj: word [i, j] has bit b set iff adj[b, i, j] != 0
    adjb4 = np.packbits(np.asarray(adj) != 0, axis=0, bitorder='little')
    adjb = np.ascontiguousarray(adjb4.transpose(1, 2, 0)).view(
        np.uint32)[:, :, 0].astype(np.int32)
    nn = np.asarray(num_nodes).astype(np.int64)
    mask = (np.arange(N)[None, :] < nn[:, None]).astype(np.float32)
    return h, edge_attr, adjb, mask


def kernel(h, edge_attr, adj, num_nodes, W_w, W_b, U_w, U_b):
    h, edge_attr, adjb, mask = _host_prep(h, edge_attr, adj, num_nodes)
    ww = np.ascontiguousarray(np.asarray(W_w, dtype=np.float32))
    wb = np.asarray(W_b, dtype=np.float32).reshape(1, H)
    uwm = np.ascontiguousarray(np.asarray(U_w, dtype=np.float32))
    ub = np.asarray(U_b, dtype=np.float32).reshape(1, H)
    ident = np.eye(128, dtype=np.float32)
    ones = np.ones((128, 1), dtype=np.float32)
    sel8 = np.tile(np.eye(8, dtype=np.float32).reshape(1, 64), (128, 1))

    nc = build_nc(reps=1,
                  variant=os.environ.get("KERNEL_VARIANT", "fast"))
    in_maps = []
    for core in range(N_CORES):
        sl = slice(core * BL, (core + 1) * BL)
        in_maps.append({
            "h": h[sl], "ea": edge_attr[sl], "adjb": adjb,
            "mask": mask[sl], "ww": ww, "wb": wb, "uw": uwm, "ub": ub,
            "ident": ident, "ones": ones, "sel8": sel8,
            "sel16": np.tile(np.eye(16, dtype=np.float32).reshape(1, 256),
                             (128, 1)),
        })
    res = run_bass_kernel_spmd(nc, in_maps, list(range(N_CORES)))
    out = np.empty((B, N, H), dtype=np.float32)
    for core in range(N_CORES):
        out[core * BL:(core + 1) * BL] = res.results[core]["y"]
    return out



# revision 4
# speedup vs baseline: 3.8714x; 3.8714x over previous
"""DMPNN layer kernel for Trainium2, data-parallel over batch on 8 NeuronCores.

Math (reference):
    gate[i,j]  = (sum_b adj[b,i,j]) > 0                      [N,N], shared across batch
    hW[b,i,o]  = sum_c h[b,i,c] * Wh[o,c]                    Wh = W_w[:, :H]
    term_h     = sum_i gate[i,j] * hW[b,i,o]
    e_sum      = sum_i gate[i,j] * edge_attr[b,i,j,e]
    term_e     = sum_e e_sum[b,j,e] * We[o,e]                We = W_w[:, H:]
    count[j]   = sum_i gate[i,j]
    msg        = term_h + term_e + count[j]*W_b[o]
    msg       *= (j < num_nodes[b])
    h_new      = (h + msg) @ U_w.T + U_b

Design (per core, BL = 4 batches; target_regime = memory):
  - edge_attr is the dominant HBM stream.  It is cast host-side to fp8
    (e4m3, "fp8" variant) or bf16 ("bf16" variant): rel tolerance is 2e-2
    and the edge contribution is diluted through We/U_w, so fp8 costs only
    ~1e-3 output error while halving/quartering HBM traffic vs f32.
  - gate is computed on device from the host bit-packed adj words
    (word[i,j] has bit b set iff adj[b,i,j] != 0  ->  any-over-batch is a
    single != 0 compare; no collective needed since every core reads the
    256 KB word matrix).
  - gating of the edge stream is a bitwise AND on a uint16 view of the
    fp8 pairs with a 0xFFFF/0x0000 mask (exact zeroing, 2x DVE mode), or
    a bf16 multiply by a 0/1 mask for the bf16 variant.
  - the i-reduction of the gated stream runs on the PE: per (batch, e)
    one fp8 DoubleRow matmul contracts both 128-row i-chunks at once
    (sel[k, c, m] = 1[m == e]) into es[e, j]; bf16 variant uses one
    matmul per (chunk, e).
  - everything is kept feature-major ("T" layout, [hidden, nodes]); h and
    the weight transposes are prepared host-side so no on-device
    transposes are needed.  y is written back transposed bf16 and
    un-transposed on host.
  - per-batch stages are software-pipelined with a skew of one batch so
    each in-order engine queue (PE / DVE / Act / Pool / SP-DMA) stays
    busy: AND(b+1) is issued before the msg/up tail of batch b.
"""

import os
import sys

for _p in ("/opt/trn_rl_repo", "/root/.axon_site/_ro/trn_rl_repo"):
    if _p not in sys.path:
        sys.path.insert(0, _p)

import numpy as np

import concourse.bass as bass
import concourse.tile as tile
from concourse import bacc, mybir
from concourse.bass_utils import run_bass_kernel_spmd

B, N, H, E = 32, 256, 128, 16
N_CORES = 8
BL = B // N_CORES          # batches per core
NJE = N * E                # 4096
F32 = mybir.dt.float32
BF16 = mybir.dt.bfloat16
U16 = mybir.dt.uint16
FP8 = mybir.dt.float8e4
I32 = mybir.dt.int32
AOP = mybir.AluOpType


def build_nc(reps: int = 1, variant: str = "fp8"):
    """variant: "fp8"  - edge stream in fp8 e4m3, AND-gating on uint16 view,
                         DoubleRow est matmuls (both i-chunks per matmul)
                "bf16" - edge stream in bf16, multiply-gating, per-chunk est
    """
    fp8 = variant == "fp8"
    EDT = FP8 if fp8 else BF16           # edge dtype
    ECOLS = 2 * NJE                      # 8192 edge elems per partition row
    MCOLS = ECOLS // 2 if fp8 else ECOLS  # u16 AND-mask cols (fp8 pairs)
    MREP = (E // 2) if fp8 else E        # mask replication per j

    nc = bacc.Bacc("TRN2", target_bir_lowering=False, debug=False,
                   num_devices=N_CORES)

    d_ht = nc.dram_tensor("ht", [BL, H, N], BF16, kind="ExternalInput")
    d_ea = nc.dram_tensor("ea", [BL, N, NJE], EDT, kind="ExternalInput")
    # adj bit-packed host-side (lossless): bit b of word [i, j] is
    # adj[b, i, j] != 0; any-over-batch == word != 0.
    d_adj = nc.dram_tensor("adjb", [N, N], I32, kind="ExternalInput")
    d_mask = nc.dram_tensor("mask", [1, BL * N], BF16, kind="ExternalInput")
    d_whT = nc.dram_tensor("whT", [H, H], BF16, kind="ExternalInput")
    d_weT = nc.dram_tensor("weT", [E, H], BF16, kind="ExternalInput")
    d_uwT = nc.dram_tensor("uwT", [H, H], BF16, kind="ExternalInput")
    d_wb = nc.dram_tensor("wb", [1, H], BF16, kind="ExternalInput")
    d_ubc = nc.dram_tensor("ubc", [H, 1], F32, kind="ExternalInput")
    d_ones = nc.dram_tensor("ones", [H, 1], BF16, kind="ExternalInput")
    # DoubleRow selectors: sel[k, 32*e + 16*c + m] = 1[m == e]  (fp8)
    # plain selectors:     sel[k, 16*e + m]        = 1[m == e]  (bf16)
    d_sel = nc.dram_tensor("sel", [128, 32 * E if fp8 else 16 * E], EDT,
                           kind="ExternalInput")
    d_y = nc.dram_tensor("y", [BL, H, N], BF16, kind="ExternalOutput")

    with tile.TileContext(nc) as tc:
        with (
            tc.tile_pool(name="const", bufs=1) as cpool,
            tc.tile_pool(name="gatep", bufs=1) as gpool,
            tc.tile_pool(name="ea", bufs=4) as eapool,
            tc.tile_pool(name="work", bufs=2) as wpool,
            tc.tile_pool(name="ps_es", bufs=2, space="PSUM") as ps_es,
            tc.tile_pool(name="ps_hw", bufs=2, space="PSUM") as ps_hw,
            tc.tile_pool(name="ps_msg", bufs=2, space="PSUM") as ps_msg,
            tc.tile_pool(name="ps_up", bufs=1, space="PSUM") as ps_up,
            tc.tile_pool(name="ps_cnt", bufs=1, space="PSUM") as ps_cnt,
        ):
            # ---- constants (preamble, not per-rep) -------------------------
            whT = cpool.tile([H, H], BF16)
            nc.sync.dma_start(whT[:], d_whT[:])
            weT = cpool.tile([E, H], BF16)
            nc.sync.dma_start(weT[:], d_weT[:])
            uwT = cpool.tile([H, H], BF16)
            nc.sync.dma_start(uwT[:], d_uwT[:])
            wb = cpool.tile([1, H], BF16)
            nc.sync.dma_start(wb[:], d_wb[:])
            ubc = cpool.tile([H, 1], F32)
            nc.sync.dma_start(ubc[:], d_ubc[:])
            ones = cpool.tile([H, 1], BF16)
            nc.sync.dma_start(ones[:], d_ones[:])
            sel = cpool.tile([128, 32 * E if fp8 else 16 * E], EDT)
            nc.sync.dma_start(sel[:], d_sel[:])

            for rep in range(reps):
                # ---- gate from packed adj words ------------------------
                at = gpool.tile([128, 2 * N], I32, name="at")
                nc.sync.dma_start(
                    at[:].rearrange("p (c j) -> p c j", c=2),
                    d_adj[:].rearrange("(c p) j -> p c j", c=2))
                # hT for all 4 batches in one DMA
                hT = gpool.tile([H, BL * N], BF16, name="hT")
                nc.sync.dma_start(
                    hT[:].rearrange("p (b j) -> p b j", b=BL),
                    d_ht[:].rearrange("b p j -> p b j"))
                # node masks, one row DMA, broadcast per batch on Pool
                mrows = gpool.tile([1, BL * N], BF16, name="mrows")
                nc.scalar.dma_start(mrows[:], d_mask[:])
                maskb = []
                for b in range(BL):
                    mb = gpool.tile([128, N], BF16, name=f"maskb{b}")
                    nc.gpsimd.partition_broadcast(mb[:],
                                                  mrows[0:1, bass.ts(b, N)])
                    maskb.append(mb)

                g = []
                for c in range(2):
                    gc = gpool.tile([128, N], BF16, name=f"g{c}")
                    nc.vector.tensor_scalar(gc[:], at[:, bass.ts(c, N)],
                                            0, None, AOP.not_equal)
                    g.append(gc)
                if fp8:
                    m16 = gpool.tile([128, 2 * N], U16, name="m16")
                    nc.vector.tensor_scalar(m16[:], at[:], 0, 65535,
                                            AOP.not_equal, AOP.mult)
                # gating mask over the full (c, j, e) edge row
                mbc = gpool.tile([128, MCOLS], U16 if fp8 else BF16,
                                 name="mbc")
                mv = mbc[:].rearrange("p (c j e) -> p c j e", c=2, e=MREP)
                if fp8:
                    src0 = m16[:, 0:N].unsqueeze(2).broadcast_to(
                        [128, N, MREP])
                    src1 = m16[:, N:2 * N].unsqueeze(2).broadcast_to(
                        [128, N, MREP])
                else:
                    src0 = g[0][:].unsqueeze(2).broadcast_to([128, N, MREP])
                    src1 = g[1][:].unsqueeze(2).broadcast_to([128, N, MREP])
                nc.scalar.copy(mv[:, 0], src0)
                nc.gpsimd.tensor_copy(mv[:, 1], src1)

                # count[j] = sum_i gate[i, j]
                cnt_ps = ps_cnt.tile([1, N], F32, name="cnt")
                for c in range(2):
                    nc.tensor.matmul(cnt_ps[:], ones[:], g[c][:],
                                     start=(c == 0), stop=(c == 1))
                cnt = gpool.tile([1, N], BF16, name="cnt_sb")
                nc.scalar.copy(cnt[:], cnt_ps[:])

                # ---- stream all 4 batches of edges up front ----------------
                ea_t = []
                for b in range(BL):
                    et = eapool.tile([128, ECOLS], EDT, name="ea_t")
                    nc.sync.dma_start(
                        et[:].rearrange("p (c je) -> p c je", c=2),
                        d_ea[b].rearrange("(c p) je -> p c je", c=2))
                    ea_t.append(et)

                # ---- software-pipelined per-batch stages -------------------
                def stage_head(b):
                    """gate the edge stream, hW, est reduction (PE-heavy)."""
                    et = ea_t[b]
                    if fp8:
                        e16 = et[:].bitcast(U16)
                        nc.vector.tensor_tensor(e16, e16, mbc[:],
                                                AOP.bitwise_and)
                    else:
                        nc.vector.tensor_tensor(et[:], et[:], mbc[:],
                                                AOP.mult)

                    hw_ps = ps_hw.tile([128, 2 * H], F32, name="hw_ps")
                    for c in range(2):
                        nc.tensor.matmul(
                            hw_ps[:, bass.ts(c, H)],
                            hT[:, b * N + 128 * c:b * N + 128 * (c + 1)],
                            whT[:], start=True, stop=True)
                    hw = wpool.tile([128, 2 * H], BF16, name="hw")
                    nc.scalar.copy(hw[:], hw_ps[:])

                    es_ps = ps_es.tile([E, N], F32, name="es_ps")
                    eav = et[:].rearrange("p (c j e) -> p c j e", c=2, e=E)
                    if fp8:
                        selv = sel[:].rearrange("p (e c m) -> p e c m",
                                                c=2, m=E)
                        for e in range(E):
                            nc.tensor.matmul(
                                es_ps[:], selv[:, e], eav[:, :, :, e],
                                start=(e == 0), stop=(e == E - 1),
                                perf_mode=mybir.MatmulPerfMode.DoubleRow)
                    else:
                        for c in range(2):
                            for e in range(E):
                                nc.tensor.matmul(
                                    es_ps[:], sel[:, bass.ts(e, E)],
                                    eav[:, c, :, e],
                                    start=(c == 0 and e == 0),
                                    stop=(c == 1 and e == E - 1))
                    esT = wpool.tile([E, N], BF16, name="esT")
                    nc.scalar.copy(esT[:], es_ps[:])
                    return hw, esT

                def stage_tail(b, hw, esT):
                    """msg accumulation, mask+h, up-projection, store."""
                    msg_ps = ps_msg.tile([H, N], F32, name="msg_ps")
                    for c in range(2):
                        nc.tensor.matmul(msg_ps[:], hw[:, bass.ts(c, H)],
                                         g[c][:], start=(c == 0), stop=False)
                    nc.tensor.matmul(msg_ps[:], wb[:], cnt[:],
                                     start=False, stop=False)
                    nc.tensor.matmul(msg_ps[:], weT[:], esT[:],
                                     start=False, stop=True)

                    xT = wpool.tile([H, N], BF16, name="xT")
                    nc.vector.tensor_tensor(xT[:], msg_ps[:], maskb[b][:],
                                            AOP.mult)
                    nc.vector.tensor_tensor(xT[:], xT[:],
                                            hT[:, bass.ts(b, N)], AOP.add)

                    up_ps = ps_up.tile([H, N], F32, name="up_ps")
                    nc.tensor.matmul(up_ps[:], uwT[:], xT[:],
                                     start=True, stop=True)
                    yt = wpool.tile([H, N], BF16, name="yt")
                    nc.scalar.activation(yt[:], up_ps[:],
                                         mybir.ActivationFunctionType.Identity,
                                         bias=ubc[:])
                    nc.scalar.dma_start(d_y[b], yt[:])

                prev = None
                for b in range(BL):
                    cur = stage_head(b)
                    if prev is not None:
                        stage_tail(b - 1, *prev)
                    prev = cur
                stage_tail(BL - 1, *prev)

    nc.compile()
    return nc


def prep_inputs(h, edge_attr, adj, num_nodes, W_w, W_b, U_w, U_b,
                variant: str = "fp8"):
    """Host-side prep: dtype casts, transposes, adj bit-packing.  Returns a
    dict of full arrays keyed by dram tensor name; index 0 is the shard dim
    for per-core arrays, others are replicated."""
    edt = mybir.dt.np(FP8 if variant == "fp8" else BF16)
    bf = mybir.dt.np(BF16)
    hT = np.ascontiguousarray(
        np.asarray(h, dtype=np.float32).transpose(0, 2, 1)).astype(bf)
    ea = np.asarray(edge_attr, dtype=np.float32).reshape(B, N, NJE).astype(edt)
    adjb4 = np.packbits(np.asarray(adj) != 0, axis=0, bitorder='little')
    adjb = np.ascontiguousarray(adjb4.transpose(1, 2, 0)).view(
        np.uint32)[:, :, 0].astype(np.int32)
    nn = np.asarray(num_nodes).astype(np.int64)
    mask = (np.arange(N)[None, :] < nn[:, None]).astype(bf).reshape(
        N_CORES, 1, BL * N)
    ww = np.asarray(W_w, dtype=np.float32)
    sel_eye = np.eye(E, dtype=np.float32)
    if variant == "fp8":
        # sel[k, 32e + 16c + m] = 1[m == e]
        sel = np.tile(np.stack([sel_eye, sel_eye], axis=1).reshape(1, 32 * E),
                      (128, 1)).astype(edt)
    else:
        sel = np.tile(sel_eye.reshape(1, 16 * E), (128, 1)).astype(edt)
    return {
        "ht": hT, "ea": ea, "adjb": adjb, "mask": mask,
        "whT": np.ascontiguousarray(ww[:, :H].T).astype(bf),
        "weT": np.ascontiguousarray(ww[:, H:].T).astype(bf),
        "uwT": np.ascontiguousarray(np.asarray(U_w, np.float32).T).astype(bf),
        "wb": np.asarray(W_b, np.float32).reshape(1, H).astype(bf),
        "ubc": np.asarray(U_b, np.float32).reshape(H, 1),
        "ones": np.ones((H, 1), dtype=bf),
        "sel": sel,
    }


SHARDED = ("ht", "ea", "mask")


def shard(full, core):
    out = {}
    for k, v in full.items():
        if k in ("ht", "ea"):
            out[k] = v[core * BL:(core + 1) * BL]
        elif k == "mask":
            out[k] = v[core]
        else:
            out[k] = v
    return out


def kernel(h, edge_attr, adj, num_nodes, W_w, W_b, U_w, U_b):
    variant = os.environ.get("KERNEL_VARIANT", "fp8")
    full = prep_inputs(h, edge_attr, adj, num_nodes, W_w, W_b, U_w, U_b,
                       variant)
    nc = build_nc(reps=1, variant=variant)
    in_maps = [shard(full, core) for core in range(N_CORES)]
    res = run_bass_kernel_spmd(nc, in_maps, list(range(N_CORES)))
    out = np.empty((B, N, H), dtype=np.float32)
    for core in range(N_CORES):
        yt = np.asarray(res.results[core]["y"]).astype(np.float32)
        out[core * BL:(core + 1) * BL] = yt.transpose(0, 2, 1)
    return out
